# revision 1
# baseline (speedup 1.0000x reference)
"""BitNet SwiGLU MLP kernel for Trainium2, tensor-parallel over 8 NeuronCores.

Sharding (Megatron-style TP over the intermediate dim F):
- Each core holds a 1/8 column-shard of Wg/Wu (fed transposed: [D, FS]) and
  the matching shard of Wd (fed as Wd[:, shard].T = [FS, D]). x is replicated,
  fed both natural-sliced (per-token quant stats, sharded over tokens) and
  fully transposed [D, T] (matmul operand layout).
- bit_linear runs as an exact integer matmul in bf16: quantized activations
  are ints in [-128,127] and ternary weights in {-1,0,1} (both exact in bf16,
  accumulated exactly in fp32 PSUM), dequantized on the output by per-token /
  global scales. clip(round(.)) for activations is exact RNE via the fp32
  magic-number trick (the clip provably never binds since |x*scale| <= 127).
- RMSNorm + requant stats (sum h^2, absmax) are computed per f-shard and
  combined with NSPLIT token-segment AllGathers so each segment's post-stats
  pass overlaps the matmuls of later segments.
- Down-proj + mean-pool is collapsed algebraically:
  mean_{h,d}(hq @ Wdq.T) = 1/(H*D) * sum_f (sum_h hq[t,f]) * (sum_d Wdq[d,f])
  so only a weighted row-reduction against S[f] = colsum(Wdq) remains. All Wd
  processing (stats, AllReduce of its abs-sum, ternary quant, row sums) is
  deferred past the main matmul loop emission and kept off the PE (partition
  reductions/broadcasts via gpsimd.partition_all_reduce) so it overlaps the
  compute instead of lengthening the kernel head.
- Pooled partials are AllReduced; every core runs the tiny classifier.
"""
import numpy as np

MAGIC = 12582912.0  # 1.5 * 2^23, fp32 RNE magic
EPS = 1e-6
QEPS = 1e-5


def build(B=8, C=3, H=128, D=2048, F=8192, NCLS=1000, NCORES=8, GSZ=4,
          NSPLIT=4, ln_is_ones=True, mock_collectives=False, use_hw_silu=True):
    """Build + compile the SPMD Bass program. Returns (nc, meta)."""
    import concourse.bacc as bacc
    import concourse.tile as tile
    from concourse import mybir
    from concourse import bass_isa

    f32 = mybir.dt.float32
    bf16 = mybir.dt.bfloat16
    AX = mybir.AxisListType
    OP = mybir.AluOpType
    AF = mybir.ActivationFunctionType
    RO = bass_isa.ReduceOp
    RG = [list(range(NCORES))]

    assert H == 128
    T = B * C * H
    TT = T // 128              # token tiles (== B*C)
    TS = T // NCORES           # tokens per core for stats
    assert TS % 128 == 0
    TST = TS // 128
    FS = F // NCORES           # f-shard width
    DT = D // 128              # contraction tiles
    NF = min(512, FS)
    FH = FS // NF
    assert TT % GSZ == 0
    NGRP = TT // GSZ
    GW = GSZ * 128
    # stats segments: uneven — big early segments overlap the matmul loop,
    # a tiny final segment minimizes the unavoidable post-loop tail
    if NSPLIT >= TT:
        SEGB = list(range(TT + 1))
    elif TT >= 12 and NSPLIT == 4:
        q = TT // 4
        SEGB = [0, q, 2 * q, 3 * q, TT - 1, TT]
    else:
        assert TT % NSPLIT == 0
        s = TT // NSPLIT
        SEGB = [0] + [(i + 1) * s for i in range(NSPLIT)]
    NSEG = len(SEGB) - 1
    SEG_OF = {}
    for si in range(NSEG):
        for t in range(SEGB[si], SEGB[si + 1]):
            SEG_OF[t] = si
    CH = min(1024, max(D, FS))

    nc = bacc.Bacc("TRN2", target_bir_lowering=False, debug=False,
                   num_devices=1 if mock_collectives else NCORES)

    def collective(kind, op, in_ap, out_ap):
        if NCORES == 1 or mock_collectives:
            n = out_ap.size() // in_ap.size()
            flat = out_ap.rearrange("a b -> (a b)")
            for r in range(n):
                nc.sync.dma_start(
                    flat[r * in_ap.size():(r + 1) * in_ap.size()], in_ap)
        else:
            nc.gpsimd.collective_compute(kind, op, replica_groups=RG,
                                         ins=[in_ap.opt()], outs=[out_ap.opt()])

    xs_t = nc.dram_tensor("xs", [TS, D], f32, kind="ExternalInput")
    xT_t = nc.dram_tensor("xT", [D, T], f32, kind="ExternalInput")
    wgT_t = nc.dram_tensor("wgT", [D, FS], f32, kind="ExternalInput")
    wuT_t = nc.dram_tensor("wuT", [D, FS], f32, kind="ExternalInput")
    wdT_t = nc.dram_tensor("wdT", [FS, D], f32, kind="ExternalInput")
    lnw_t = nc.dram_tensor("lnw", [1, FS], f32, kind="ExternalInput")
    clsWT_t = nc.dram_tensor("clsWT", [C, NCLS], f32, kind="ExternalInput")
    clsb_t = nc.dram_tensor("clsb", [1, NCLS], f32, kind="ExternalInput")
    out_t = nc.dram_tensor("out", [B, NCLS], f32, kind="ExternalOutput")

    def r128(ap):
        # [1, n*128] dram view -> [128, n] (partition = fast axis)
        return ap.rearrange("o (i p) -> (o p) i", p=128)

    with tile.TileContext(nc) as tc:
        import contextlib
        with contextlib.ExitStack() as st:
            dram = st.enter_context(tc.tile_pool(name="dram", bufs=1, space="DRAM"))
            sbC = st.enter_context(tc.tile_pool(name="sbC", bufs=1))
            sbS4 = st.enter_context(tc.tile_pool(name="sbS4", bufs=5))
            sbS8 = st.enter_context(tc.tile_pool(name="sbS8", bufs=2))
            sbS2 = st.enter_context(tc.tile_pool(name="sbS2", bufs=5))
            sbSG = st.enter_context(tc.tile_pool(name="sbSG", bufs=2))
            sbR5 = st.enter_context(tc.tile_pool(name="sbR5", bufs=3))
            sbH = st.enter_context(tc.tile_pool(name="sbH", bufs=3))
            sbXQ = st.enter_context(
                tc.tile_pool(name="sbXQ", bufs=DT + DT // 2))
            sbWQ = st.enter_context(tc.tile_pool(name="sbWQ", bufs=2 * DT))
            sbCol = st.enter_context(tc.tile_pool(name="sbCol", bufs=2))

            h_dram = dram.tile([T, FS], f32)
            c1_in = dram.tile([1, 8], f32)
            c1_out = dram.tile([1, 8], f32)
            c2_in = dram.tile([1, 8], f32)
            c2_out = dram.tile([1, 8], f32)
            sc_in = dram.tile([1, TS], f32)
            sc_out = dram.tile([1, T], f32)
            srow_dram = dram.tile([1, FS], f32)
            st_in = [dram.tile([2, (SEGB[i + 1] - SEGB[i]) * 128], f32,
                                name=f"st_in{i}") for i in range(NSEG)]
            st_out = [dram.tile([NCORES * 2, (SEGB[i + 1] - SEGB[i]) * 128],
                                f32, name=f"st_out{i}") for i in range(NSEG)]
            pl_in = dram.tile([1, TT], f32)
            pl_out = dram.tile([1, TT], f32)

            ones1 = sbC.tile([1, 128], f32)
            nc.vector.memset(ones1[:], 1.0)
            negmagic = sbC.tile([128, 1], f32)
            nc.vector.memset(negmagic[:], -MAGIC)

            # ================= Phase 0: Wg/Wu + x abs-stats =================
            NW = 2 * DT * ((FS + CH - 1) // CH) + TST * ((D + CH - 1) // CH)
            wacc = sbC.tile([128, NW], f32)
            col = 0
            wcols = {}
            # x-slice per-token clipped absmax -> sc_in -> AllGather
            for i in range(TST):
                xt = sbS8.tile([128, D], f32, tag="S8")
                nc.sync.dma_start(xt[:], xs_t.ap()[i * 128:(i + 1) * 128, :])
                am = sbCol.tile([128, 1], f32, tag="am")
                nc.vector.tensor_reduce(out=am[:], in_=xt[:], axis=AX.X,
                                        op=OP.max, apply_absolute_value=True)
                amc = sbCol.tile([128, 1], f32, tag="amc")
                nc.vector.tensor_scalar(out=amc[:], in0=am[:], scalar1=QEPS,
                                        scalar2=None, op0=OP.max)
                nc.sync.dma_start(sc_in[0:1, i * 128:(i + 1) * 128], amc[:])
            collective("AllGather", OP.bypass, sc_in[:], sc_out[:])
            WB = 2 if DT % 2 == 0 else 1   # d-blocks per weight DMA
            for name, ten, ntile, width in (("wg", wgT_t, DT, FS),
                                            ("wu", wuT_t, DT, FS)):
                c0 = col
                for i in range(0, ntile, WB):
                    wt = sbS8.tile([128, WB * width], f32, tag="S8")
                    nc.sync.dma_start(
                        wt[:],
                        ten.ap()[i * 128:(i + WB) * 128, :]
                        .rearrange("(b p) c -> p b c", p=128))
                    nc.vector.tensor_reduce(
                        out=wacc[:, col:col + 1], in_=wt[:], axis=AX.X,
                        op=OP.add, apply_absolute_value=True)
                    col += 1
                wcols[name] = (c0, col)
            c1_sb = sbC.tile([1, 8], f32)
            nc.vector.memset(c1_sb[:], 0.0)
            for j, name in enumerate(("wg", "wu")):
                c0, c1c = wcols[name]
                colsum = sbCol.tile([128, 1], f32, tag="wcolsum")
                nc.vector.tensor_reduce(out=colsum[:], in_=wacc[:, c0:c1c],
                                        axis=AX.X, op=OP.add)
                csb = sbCol.tile([128, 1], f32, tag="wcolsum2")
                nc.gpsimd.partition_all_reduce(csb[:], colsum[:], channels=128,
                                               reduce_op=RO.add)
                nc.vector.tensor_copy(c1_sb[:, j:j + 1], csb[0:1, :])
            nc.sync.dma_start(c1_in[:], c1_sb[:])
            collective("AllReduce", OP.add, c1_in[:], c1_out[:])


            # ================= Phase 1: derived scalars =================
            c1g = sbC.tile([1, 8], f32)
            nc.sync.dma_start(c1g[:], c1_out[:])
            m_w = sbC.tile([1, 2], f32)
            nc.vector.tensor_scalar(out=m_w[:], in0=c1g[:, 0:2],
                                    scalar1=1.0 / (float(F) * D), scalar2=QEPS,
                                    op0=OP.mult, op1=OP.max)
            s_w = sbC.tile([1, 2], f32)
            nc.vector.reciprocal(s_w[:], m_w[:])

            m_w_col = sbC.tile([128, 3], f32)
            s_w_col = sbC.tile([128, 3], f32)
            S_bcast = sbC.tile([128, T], f32)
            Sh_bcast = sbC.tile([128, FS], f32)
            with tc.tile_pool(name="ps1", bufs=2, space="PSUM") as ps1:
                for j in range(0, T, 512):
                    w = min(512, T - j)
                    sc_c = sbR5.tile([1, 512], f32, tag="R5a")
                    nc.sync.dma_start(sc_c[:, 0:w], sc_out[0:1, j:j + w])
                    pb = ps1.tile([128, 512], f32, tag="pb")
                    nc.tensor.matmul(pb[:, 0:w], ones1[:], sc_c[:, 0:w],
                                     start=True, stop=True)
                    rw = sbS2.tile([128, 512], f32, tag="S2")
                    nc.vector.reciprocal(rw[:, 0:w], pb[:, 0:w])
                    nc.vector.tensor_scalar(out=S_bcast[:, j:j + w],
                                            in0=rw[:, 0:w], scalar1=127.0,
                                            scalar2=None, op0=OP.mult)
                pm = ps1.tile([128, 2], f32, tag="pm")
                nc.tensor.matmul(pm[:], ones1[:], m_w[:], start=True, stop=True)
                nc.vector.tensor_copy(m_w_col[:, 0:2], pm[:])
                psw = ps1.tile([128, 2], f32, tag="pm")
                nc.tensor.matmul(psw[:], ones1[:], s_w[:], start=True, stop=True)
                nc.vector.tensor_copy(s_w_col[:, 0:2], psw[:])
                if not ln_is_ones:
                    Ln_bcast = sbC.tile([128, FS], f32)
                    for j in range(0, FS, 512):
                        w = min(512, FS - j)
                        lnr = sbR5.tile([1, 512], f32, tag="R5a")
                        nc.sync.dma_start(lnr[:, 0:w], lnw_t.ap()[0:1, j:j + w])
                        pb = ps1.tile([128, 512], f32, tag="pb")
                        nc.tensor.matmul(pb[:, 0:w], ones1[:], lnr[:, 0:w],
                                         start=True, stop=True)
                        nc.vector.tensor_copy(Ln_bcast[:, j:j + w], pb[:, 0:w])

            DEQG = sbC.tile([128, TT], f32)
            DEQU = sbC.tile([128, TT], f32)
            yraw = sbC.tile([128, TT], f32)
            nc.sync.dma_start(yraw[:], r128(sc_out[:]))
            nc.vector.tensor_scalar(out=DEQG[:], in0=yraw[:],
                                    scalar1=m_w_col[:, 0:1], scalar2=1.0 / 127.0,
                                    op0=OP.mult, op1=OP.mult)
            nc.vector.tensor_scalar(out=DEQU[:], in0=yraw[:],
                                    scalar1=m_w_col[:, 1:2], scalar2=1.0 / 127.0,
                                    op0=OP.mult, op1=OP.mult)

            # ================= Phase 2: Wg/Wu quantization =================
            wq_g, wq_u = [], []
            for i in range(0, DT, WB):
                for lst, ten, scol in ((wq_g, wgT_t, 0), (wq_u, wuT_t, 1)):
                    wt = sbS8.tile([128, WB * FS], f32, tag="S8")
                    nc.sync.dma_start(
                        wt[:],
                        ten.ap()[i * 128:(i + WB) * 128, :]
                        .rearrange("(b p) c -> p b c", p=128))
                    for b in range(WB):
                        t1 = sbS4.tile([128, FS], f32, tag="S4")
                        nc.scalar.activation(out=t1[:],
                                             in_=wt[:, b * FS:(b + 1) * FS],
                                             func=AF.Copy,
                                             scale=s_w_col[:, scol:scol + 1],
                                             bias=MAGIC)
                        t2 = sbS4.tile([128, FS], f32, tag="S4")
                        nc.gpsimd.tensor_scalar(out=t2[:], in0=t1[:],
                                                scalar1=MAGIC, scalar2=1.0,
                                                op0=OP.subtract, op1=OP.min)
                        wq = sbWQ.tile([128, FS], bf16, tag="wq")
                        nc.vector.tensor_scalar(out=wq[:], in0=t2[:],
                                                scalar1=-1.0, scalar2=None,
                                                op0=OP.max)
                        lst.append(wq)

            stat2 = sbC.tile([128, 2 * TT], f32)
            Q_cols = sbC.tile([128, TT], f32)

            NCHD = (D + CH - 1) // CH
            wd_acc = sbC.tile([128, FS // 128], f32)

            # Deferred Wd processing: interleaved into the matmul group loop
            # so its DMA/compute overlaps the MMs; uses no PE (partition
            # reductions/broadcasts go through gpsimd.partition_all_reduce)
            # so it cannot stall the in-order matmul stream.
            def emit_wd_stats():
                wcol = 0
                for i in range(FS // 128):
                    wt0 = sbS8.tile([128, D], f32, tag="S8")
                    nc.sync.dma_start(wt0[:], wdT_t.ap()[i * 128:(i + 1) * 128, :])
                    nc.vector.tensor_reduce(
                        out=wd_acc[:, wcol:wcol + 1], in_=wt0[:],
                        axis=AX.X, op=OP.add, apply_absolute_value=True)
                    wcol += 1
            def emit_wd_scalars():
                wdsum = sbCol.tile([128, 1], f32, tag="wdsum")
                nc.vector.tensor_reduce(out=wdsum[:], in_=wd_acc[:], axis=AX.X,
                                        op=OP.add)
                wdtot = sbCol.tile([128, 1], f32, tag="wdtot")
                nc.gpsimd.partition_all_reduce(wdtot[:], wdsum[:], channels=128,
                                               reduce_op=RO.add)
                c2_sb = sbC.tile([1, 8], f32)
                nc.vector.memset(c2_sb[:], 0.0)
                nc.vector.tensor_copy(c2_sb[:, 0:1], wdtot[0:1, :])
                nc.sync.dma_start(c2_in[:], c2_sb[:])
                collective("AllReduce", OP.add, c2_in[:], c2_out[:])
                c2g = sbC.tile([1, 8], f32)
                nc.sync.dma_start(c2g[:], c2_out[:])
                mws_pad = sbC.tile([128, 2], f32)
                nc.vector.memset(mws_pad[:], 0.0)
                nc.vector.tensor_scalar(out=mws_pad[0:1, 0:1], in0=c2g[:, 0:1],
                                        scalar1=1.0 / (float(F) * D), scalar2=QEPS,
                                        op0=OP.mult, op1=OP.max)
                nc.vector.reciprocal(mws_pad[0:1, 1:2], mws_pad[0:1, 0:1])
                mws_col = sbC.tile([128, 2], f32)
                nc.gpsimd.partition_all_reduce(mws_col[:], mws_pad[:], channels=128,
                                               reduce_op=RO.add)
                nc.vector.tensor_copy(m_w_col[:, 2:3], mws_col[:, 0:1])
                nc.vector.tensor_copy(s_w_col[:, 2:3], mws_col[:, 1:2])
            def emit_wd_quant():
                # ternary quant + row sums -> S[f] (exact small ints)
                Ssh_cols = sbC.tile([128, max(1, FS // 128)], f32)
                for i in range(FS // 128):
                    parts = sbCol.tile([128, NCHD], f32, tag="wdparts")
                    wt = sbS8.tile([128, D], f32, tag="S8")
                    nc.sync.dma_start(wt[:], wdT_t.ap()[i * 128:(i + 1) * 128, :])
                    for j in range(0, D, CH):
                        w = min(CH, D - j)
                        t1 = sbS4.tile([128, CH], f32, tag="S4")
                        nc.scalar.activation(out=t1[:, 0:w], in_=wt[:, j:j + w],
                                             func=AF.Copy, scale=s_w_col[:, 2:3],
                                             bias=MAGIC)
                        t2 = sbS4.tile([128, CH], f32, tag="S4")
                        nc.gpsimd.tensor_scalar(out=t2[:, 0:w], in0=t1[:, 0:w],
                                                scalar1=MAGIC, scalar2=1.0,
                                                op0=OP.subtract, op1=OP.min)
                        t3 = sbS4.tile([128, CH], f32, tag="S4")
                        nc.vector.tensor_scalar(out=t3[:, 0:w], in0=t2[:, 0:w],
                                                scalar1=-1.0, scalar2=None,
                                                op0=OP.max)
                        nc.vector.tensor_reduce(
                            out=parts[:, j // CH:j // CH + 1], in_=t3[:, 0:w],
                            axis=AX.X, op=OP.add)
                    nc.vector.tensor_reduce(out=Ssh_cols[:, i:i + 1], in_=parts[:],
                                            axis=AX.X, op=OP.add)
                # S row -> broadcast down partitions (no PE): zero-pad + par-reduce
                nc.sync.dma_start(r128(srow_dram[:]), Ssh_cols[:])
                shpad = sbS4.tile([128, FS], f32, tag="S4")
                nc.vector.memset(shpad[:], 0.0)
                nc.sync.dma_start(shpad[0:1, :], srow_dram[:])
                nc.gpsimd.partition_all_reduce(Sh_bcast[:], shpad[:], channels=128,
                                               reduce_op=RO.add)
            def emit_post_stats(seg):
                SEG = SEGB[seg + 1] - SEGB[seg]
                t0 = SEGB[seg]
                stout = st_out[seg]
                ssq_g = sbSG.tile([128, SEG, NCORES], f32, tag="SG",
                                  name="ssq_g")
                gm2_g = sbSG.tile([128, SEG, NCORES], f32, tag="SG",
                                  name="gm2_g")
                for r in range(NCORES):
                    nc.sync.dma_start(
                        ssq_g[:, :, r:r + 1],
                        r128(stout[2 * r:2 * r + 1, :])[:, :, None])
                    nc.sync.dma_start(
                        gm2_g[:, :, r:r + 1],
                        r128(stout[2 * r + 1:2 * r + 2, :])[:, :, None])
                ssq12 = sbCol.tile([128, SEG], f32, tag="st_a")
                nc.vector.tensor_reduce(out=ssq12[:], in_=ssq_g[:], axis=AX.X,
                                        op=OP.add)
                gm212 = sbCol.tile([128, SEG], f32, tag="st_b")
                nc.vector.tensor_reduce(out=gm212[:], in_=gm2_g[:], axis=AX.X,
                                        op=OP.max)
                v = sbCol.tile([128, SEG], f32, tag="st_c")
                nc.vector.tensor_scalar(out=v[:], in0=ssq12[:], scalar1=1.0 / F,
                                        scalar2=EPS, op0=OP.mult, op1=OP.add)
                sv = sbCol.tile([128, SEG], f32, tag="st_d")
                nc.scalar.activation(out=sv[:], in_=v[:], func=AF.Sqrt)
                rs = sbCol.tile([128, SEG], f32, tag=f"rs{seg}")
                nc.vector.reciprocal(rs[:], sv[:])
                gmax = sbCol.tile([128, SEG], f32, tag="st_e")
                if ln_is_ones:
                    nc.scalar.activation(out=gmax[:], in_=gm212[:], func=AF.Sqrt)
                else:
                    nc.vector.tensor_copy(gmax[:], gm212[:])
                rg = sbCol.tile([128, SEG], f32, tag="st_f")
                nc.vector.tensor_tensor(out=rg[:], in0=rs[:], in1=gmax[:],
                                        op=OP.mult)
                y2 = sbCol.tile([128, SEG], f32, tag="st_g")
                nc.vector.tensor_scalar(out=y2[:], in0=rg[:], scalar1=QEPS,
                                        scalar2=None, op0=OP.max)
                invs2 = sbCol.tile([128, SEG], f32, tag=f"iv{seg}")
                nc.vector.tensor_scalar(
                    out=invs2[:], in0=y2[:], scalar1=m_w_col[:, 2:3],
                    scalar2=1.0 / (127.0 * float(H) * D), op0=OP.mult,
                    op1=OP.mult)
                r2 = sbCol.tile([128, SEG], f32, tag="st_h")
                nc.vector.reciprocal(r2[:], y2[:])
                al = sbCol.tile([128, SEG], f32, tag="st_i")
                nc.vector.tensor_tensor(out=al[:], in0=r2[:], in1=rs[:],
                                        op=OP.mult)
                alpha = sbCol.tile([128, SEG], f32, tag=f"al{seg}")
                nc.vector.tensor_scalar(out=alpha[:], in0=al[:], scalar1=127.0,
                                        scalar2=None, op0=OP.mult)

                for i in range(SEG):
                    t = t0 + i
                    hr = sbH.tile([128, FS], f32, tag="h")
                    nc.gpsimd.dma_start(hr[:], h_dram[t * 128:(t + 1) * 128, :])
                    w1 = sbS4.tile([128, FS], f32, tag="S4")
                    nc.vector.tensor_scalar(out=w1[:], in0=hr[:],
                                            scalar1=alpha[:, i:i + 1],
                                            scalar2=MAGIC, op0=OP.mult,
                                            op1=OP.add)
                    hq = sbS4.tile([128, FS], f32, tag="S4")
                    nc.scalar.activation(out=hq[:], in_=w1[:], func=AF.Identity,
                                         bias=negmagic[:])
                    junk = sbS4.tile([128, FS], f32, tag="S4")
                    nc.gpsimd.tensor_tensor(out=junk[:], in0=hq[:],
                                            in1=Sh_bcast[:], op=OP.mult)
                    qacc = sbCol.tile([128, 1], f32, tag="qacc")
                    nc.vector.tensor_reduce(out=qacc[:], in_=junk[:], axis=AX.X,
                                            op=OP.add)
                    nc.vector.tensor_scalar(out=Q_cols[:, t:t + 1], in0=qacc[:],
                                            scalar1=invs2[:, i:i + 1],
                                            scalar2=None, op0=OP.mult)


            # ================= Phase 3: main matmul loop =================
            with tc.tile_pool(name="psM", bufs=2, space="PSUM") as psM:
                for g in range(NGRP):
                    xq_slabs = []
                    for d0 in range(0, DT, WB):
                        xsl = sbS4.tile([128, WB * GW], f32, tag="S4x", bufs=3)
                        nc.sync.dma_start(
                            xsl[:],
                            xT_t.ap()[d0 * 128:(d0 + WB) * 128,
                                      g * GW:(g + 1) * GW]
                            .rearrange("(b p) c -> p b c", p=128))
                        for b in range(WB):
                            prod = sbS2.tile([128, GW], f32, tag="S2")
                            nc.vector.tensor_tensor(
                                out=prod[:], in0=xsl[:, b * GW:(b + 1) * GW],
                                in1=S_bcast[:, g * GW:(g + 1) * GW], op=OP.mult)
                            xq = sbXQ.tile([128, GW], bf16, tag="xq")
                            nc.vector.tensor_scalar(out=xq[:], in0=prod[:],
                                                    scalar1=MAGIC, scalar2=MAGIC,
                                                    op0=OP.add, op1=OP.subtract)
                            xq_slabs.append(xq)

                    for tl in range(GSZ):
                        t = g * GSZ + tl
                        tc0 = tl * 128
                        gps = [psM.tile([128, NF], f32, tag=f"g{j}",
                                        name=f"gp{j}") for j in range(FH)]
                        ups = [psM.tile([128, NF], f32, tag=f"u{j}",
                                        name=f"up{j}") for j in range(FH)]
                        for d in range(DT):
                            lhsT = xq_slabs[d][:, tc0:tc0 + 128]
                            s0, s1 = (d == 0), (d == DT - 1)
                            for j in range(FH):
                                nc.tensor.matmul(
                                    gps[j][:], lhsT,
                                    wq_g[d][:, j * NF:(j + 1) * NF],
                                    start=s0, stop=s1)
                                nc.tensor.matmul(
                                    ups[j][:], lhsT,
                                    wq_u[d][:, j * NF:(j + 1) * NF],
                                    start=s0, stop=s1)
                        ht = sbH.tile([128, FS], f32, tag="h")
                        for j in range(FH):
                            us = sbS2.tile([128, NF], f32, tag="S2")
                            nc.scalar.activation(out=us[:], in_=ups[j][:],
                                                 func=AF.Copy,
                                                 scale=DEQU[:, t:t + 1])
                            if use_hw_silu:
                                gsl = sbS2.tile([128, NF], f32, tag="S2")
                                nc.scalar.activation(out=gsl[:], in_=gps[j][:],
                                                     func=AF.Silu,
                                                     scale=DEQG[:, t:t + 1])
                            else:
                                gsg = sbS2.tile([128, NF], f32, tag="S2")
                                nc.scalar.activation(out=gsg[:], in_=gps[j][:],
                                                     func=AF.Sigmoid,
                                                     scale=DEQG[:, t:t + 1])
                                gdq = sbS2.tile([128, NF], f32, tag="S2")
                                nc.scalar.activation(out=gdq[:], in_=gps[j][:],
                                                     func=AF.Copy,
                                                     scale=DEQG[:, t:t + 1])
                                gsl = sbS2.tile([128, NF], f32, tag="S2")
                                nc.vector.tensor_tensor(out=gsl[:], in0=gsg[:],
                                                        in1=gdq[:], op=OP.mult)
                            nc.vector.tensor_tensor(
                                out=ht[:, j * NF:(j + 1) * NF], in0=gsl[:],
                                in1=us[:], op=OP.mult)
                        hsq = sbS4.tile([128, FS], f32, tag="S4")
                        nc.scalar.activation(out=hsq[:], in_=ht[:],
                                             func=AF.Square,
                                             accum_out=stat2[:, 2 * t:2 * t + 1])
                        if ln_is_ones:
                            nc.vector.tensor_reduce(
                                out=stat2[:, 2 * t + 1:2 * t + 2], in_=hsq[:],
                                axis=AX.X, op=OP.max)
                        else:
                            h2 = sbH.tile([128, FS], f32, tag="h")
                            nc.vector.tensor_tensor(out=h2[:], in0=ht[:],
                                                    in1=Ln_bcast[:], op=OP.mult)
                            nc.vector.tensor_reduce(
                                out=stat2[:, 2 * t + 1:2 * t + 2], in_=h2[:],
                                axis=AX.X, op=OP.max,
                                apply_absolute_value=True)
                            ht = h2
                        nc.gpsimd.dma_start(
                            h_dram[t * 128:(t + 1) * 128, :], ht[:])
                        seg = SEG_OF[t]
                        toff = t - SEGB[seg]
                        nc.sync.dma_start(
                            st_in[seg][0:1, toff * 128:(toff + 1) * 128],
                            stat2[:, 2 * t:2 * t + 1])
                        nc.sync.dma_start(
                            st_in[seg][1:2, toff * 128:(toff + 1) * 128],
                            stat2[:, 2 * t + 1:2 * t + 2])
                        if t == SEGB[seg + 1] - 1:
                            collective("AllGather", OP.bypass,
                                       st_in[seg][:], st_out[seg][:])

                    # interleave deferred work so it overlaps later groups
                    if g == 0:
                        emit_wd_stats()
                    elif g == 1:
                        emit_wd_scalars()
                    elif g == 2:
                        emit_wd_quant()
                    for s in range(NSEG):
                        gdone = (SEGB[s + 1] - 1) // GSZ
                        if g == gdone + 2 and g < NGRP:
                            emit_post_stats(s)

                # segments whose slack window ran past the loop end
                for s in range(NSEG):
                    gdone = (SEGB[s + 1] - 1) // GSZ
                    if gdone + 2 >= NGRP:
                        emit_post_stats(s)

            # ============ Phase 5: pooled partials + classifier ============
            qsum = sbC.tile([128, TT], f32)
            nc.gpsimd.partition_all_reduce(qsum[:], Q_cols[:], channels=128,
                                           reduce_op=RO.add)
            nc.sync.dma_start(pl_in[:], qsum[0:1, :])
            collective("AllReduce", OP.add, pl_in[:], pl_out[:])
            with tc.tile_pool(name="psE", bufs=1, space="PSUM") as psE:
                pool3 = sbC.tile([C, B], f32)
                nc.sync.dma_start(
                    pool3[:], pl_out[:].rearrange("o (b c) -> (o c) b", c=C))
                clsW_sb = sbC.tile([C, NCLS], f32)
                nc.sync.dma_start(clsW_sb[:], clsWT_t.ap())
                clsb_sb = sbC.tile([1, NCLS], f32)
                nc.sync.dma_start(clsb_sb[:], clsb_t.ap())
                out_sb = sbC.tile([B, NCLS], f32)
                for j in range(0, NCLS, 512):
                    w = min(512, NCLS - j)
                    pcls = psE.tile([B, 512], f32, tag="pcls")
                    nc.tensor.matmul(pcls[:, 0:w], pool3[:], clsW_sb[:, j:j + w],
                                     start=True, stop=False)
                    nc.tensor.matmul(pcls[:, 0:w], ones1[:, 0:B],
                                     clsb_sb[:, j:j + w], start=False, stop=True)
                    nc.vector.tensor_copy(out_sb[:, j:j + w], pcls[:, 0:w])
                nc.sync.dma_start(out_t.ap(), out_sb[:])

    nc.compile()
    meta = dict(B=B, C=C, H=H, D=D, F=F, NCLS=NCLS, NCORES=NCORES,
                T=T, TS=TS, FS=FS)
    return nc, meta


def make_in_maps(x, Wg, Wu, Wd, ln_w, cls_W, cls_b, meta):
    """Host-side sharding: slices/transposes only, no arithmetic."""
    T, TS, FS = meta["T"], meta["TS"], meta["FS"]
    D = meta["D"]
    NCLS = meta["NCLS"]
    NCORES = meta["NCORES"]
    xf = np.ascontiguousarray(np.asarray(x, np.float32).reshape(T, D))
    xT = np.ascontiguousarray(xf.T)
    clsWT = np.ascontiguousarray(np.asarray(cls_W, np.float32).T)
    clsb2 = np.ascontiguousarray(np.asarray(cls_b, np.float32).reshape(1, NCLS))
    maps = []
    for k in range(NCORES):
        f0 = k * FS
        maps.append({
            "xs": np.ascontiguousarray(xf[k * TS:(k + 1) * TS]),
            "xT": xT,
            "wgT": np.ascontiguousarray(np.asarray(Wg, np.float32)[f0:f0 + FS, :].T),
            "wuT": np.ascontiguousarray(np.asarray(Wu, np.float32)[f0:f0 + FS, :].T),
            "wdT": np.ascontiguousarray(np.asarray(Wd, np.float32)[:, f0:f0 + FS].T),
            "lnw": np.ascontiguousarray(np.asarray(ln_w, np.float32)[f0:f0 + FS].reshape(1, FS)),
            "clsWT": clsWT,
            "clsb": clsb2,
        })
    return maps


_CACHE = {}


def kernel(x, Wg, Wu, Wd, ln_w, cls_W, cls_b):
    """Takes FULL inputs, runs the 8-core SPMD Bass kernel, returns [B, NCLS]."""
    from concourse import bass_utils

    x = np.asarray(x, np.float32)
    B, C, H, D = x.shape
    F = int(np.asarray(Wg).shape[0])
    NCLS = int(np.asarray(cls_W).shape[0])
    ln_ones = bool(np.all(np.asarray(ln_w) == 1.0))
    key = (B, C, H, D, F, NCLS, ln_ones)
    if key not in _CACHE:
        _CACHE[key] = build(B=B, C=C, H=H, D=D, F=F, NCLS=NCLS, NCORES=8,
                            ln_is_ones=ln_ones)
    nc, meta = _CACHE[key]
    in_maps = make_in_maps(x, Wg, Wu, Wd, ln_w, cls_W, cls_b, meta)
    res = bass_utils.run_bass_kernel_spmd(nc, in_maps, core_ids=list(range(8)))
    return np.asarray(res.results[0]["out"], np.float32)



# revision 23
# speedup vs baseline: 1.0394x; 1.0394x over previous
"""BitNet SwiGLU MLP kernel for Trainium2 — zero-collective data-parallel
over 8 NeuronCores.

Sharding: pure data-parallel over the batch dim. Core k handles batch row
b=k (C*H = 384 tokens = 3 token tiles) and reads the FULL weights. Every
global quantity is computed core-locally, so the kernel has NO collectives
and no cross-core sync points — each core's NEFF runs independently of
launch skew:
- per-token activation absmax: from the core's own 384 token rows.
- global weight scales 1/mean|W|: each core scans the full Wg/Wu/Wd
  (it must stream them for the matmuls anyway; one extra stats pass).
- RMSNorm + requant stats over the full intermediate F: rows are complete
  per core (no F-sharding), so stats are local.
- down-proj + mean-pool collapses algebraically (as in the TP variant):
  mean_{h,d}(hq @ Wdq.T) = 1/(H*D) * sum_f (sum_h hq[t,f]) * S[f] with
  S[f] = sum_d Wdq[d,f], so Wd is consumed as a quantize+colsum scan only.
- classifier: core k's pooled row [C] is complete locally -> it computes
  output row k. Host assembles rows (pure gather, no arithmetic).

bit_linear is an exact integer matmul in bf16 (ints |.|<=127 and ternary
weights are exact in bf16; fp32 PSUM accumulation), with fp32
magic-number RNE rounding for the quantizers. h is kept resident in SBUF
as fp16 (never spilled to DRAM). DMA ~410MB/core of weight traffic at
~360GB/s is the roofline; PE/Vector/Scalar all have slack under it.
"""
import numpy as np

MAGIC = 12582912.0  # 1.5 * 2^23, fp32 RNE magic
EPS = 1e-6
QEPS = 1e-5


def build(B=8, C=3, H=128, D=2048, F=8192, NCLS=1000, NCORES=8,
          ln_is_ones=True):
    """Build + compile the per-core Bass program. Returns (nc, meta)."""
    import concourse.bacc as bacc
    import concourse.tile as tile
    from concourse import mybir
    from concourse import bass_isa

    f32 = mybir.dt.float32
    bf16 = mybir.dt.bfloat16
    fp16 = mybir.dt.float16
    AX = mybir.AxisListType
    OP = mybir.AluOpType
    AF = mybir.ActivationFunctionType
    RO = bass_isa.ReduceOp

    assert H == 128 and B == NCORES
    TK = C * H                 # tokens per core (384)
    TT = C                     # token tiles per core (3)
    DT = D // 128              # contraction tiles (16)
    FBW = 512                  # f-block width (psum bank per (mat, t))
    NFB = F // FBW             # 16 f-blocks
    SLAB = 512                 # weight rows (d) per main-pass DMA
    NSLAB = D // SLAB          # 4 slabs per f-block
    FT = F // 128              # 64 wd row tiles

    nc = bacc.Bacc("TRN2", target_bir_lowering=False, debug=False,
                   num_devices=NCORES)

    xs_t = nc.dram_tensor("xs", [TK, D], f32, kind="ExternalInput")
    xT_t = nc.dram_tensor("xT", [D, TK], f32, kind="ExternalInput")
    # pre-tiled on host: row-block (fb*NSLAB+s)*128+p, col b*FBW+c holds
    # Wg.T[s*SLAB + b*128 + p, fb*FBW + c] -> every DMA below is a plain
    # [128, 2048] slice with 8KB contiguous per partition row
    wgf_t = nc.dram_tensor("wgf", [F, D], f32, kind="ExternalInput")
    wuf_t = nc.dram_tensor("wuf", [F, D], f32, kind="ExternalInput")
    wdT_t = nc.dram_tensor("wdT", [F, D], f32, kind="ExternalInput")
    if not ln_is_ones:
        lnw_t = nc.dram_tensor("lnw", [1, F], f32, kind="ExternalInput")
    clsWT_t = nc.dram_tensor("clsWT", [C, NCLS], f32, kind="ExternalInput")
    clsb_t = nc.dram_tensor("clsb", [1, NCLS], f32, kind="ExternalInput")
    out_t = nc.dram_tensor("out", [1, NCLS], f32, kind="ExternalOutput")

    def r128(ap):
        # [1, n*128] dram view -> [128, n] (partition = fast axis)
        return ap.rearrange("o (i p) -> (o p) i", p=128)

    with tile.TileContext(nc) as tc:
        import contextlib
        with contextlib.ExitStack() as st:
            dram = st.enter_context(tc.tile_pool(name="dram", bufs=1,
                                                 space="DRAM"))
            sbC = st.enter_context(tc.tile_pool(name="sbC", bufs=1))
            sbW = st.enter_context(tc.tile_pool(name="sbW", bufs=4))
            sbQ1 = st.enter_context(tc.tile_pool(name="sbQ1", bufs=2))
            sbQ2 = st.enter_context(tc.tile_pool(name="sbQ2", bufs=1))
            sbWQ = st.enter_context(tc.tile_pool(name="sbWQ", bufs=5))
            sbE = st.enter_context(tc.tile_pool(name="sbE", bufs=2))
            sbR = st.enter_context(tc.tile_pool(name="sbR", bufs=2))
            sbX = st.enter_context(tc.tile_pool(name="sbX", bufs=3))
            sbCol = st.enter_context(tc.tile_pool(name="sbCol", bufs=3))
            psM = st.enter_context(tc.tile_pool(name="psM", bufs=1,
                                                space="PSUM"))
            psB = st.enter_context(tc.tile_pool(name="psB", bufs=1,
                                                space="PSUM"))

            srow_dram = dram.tile([1, F], f32)
            scd_dram = dram.tile([1, TK], f32)
            pl_dram = dram.tile([1, 4], f32)

            ones1 = sbC.tile([1, 128], f32)
            nc.vector.memset(ones1[:], 1.0)
            negmagic = sbC.tile([128, 1], f32)
            nc.vector.memset(negmagic[:], -MAGIC)
            neghalf = sbC.tile([128, 1], f32)
            nc.vector.memset(neghalf[:], -0.5)
            poshalf = sbC.tile([128, 1], f32)
            nc.vector.memset(poshalf[:], 0.5)
            clsW_sb = sbC.tile([C, NCLS], f32)
            nc.scalar.dma_start(clsW_sb[:], clsWT_t.ap())
            clsb_sb = sbC.tile([1, NCLS], f32)
            nc.scalar.dma_start(clsb_sb[:], clsb_t.ap())

            # persistent state
            h_t = [sbC.tile([128, F], fp16, name=f"h{t}") for t in range(TT)]
            xq = [sbC.tile([128, TK], bf16, name=f"xq{d}") for d in range(DT)]
            S8k = sbC.tile([128, F], bf16)        # colsum(Wdq) row-bcast
            Sb384 = sbC.tile([128, TK], f32)      # 127/absmax per token
            deqb = sbC.tile([128, TT], f32)       # absmax/127 per token
            DEQG = sbC.tile([128, TT], f32)
            DEQU = sbC.tile([128, TT], f32)
            ssq_p = [sbC.tile([128, NFB], f32, name=f"ssq{t}")
                     for t in range(TT)]
            m2_p = [sbC.tile([128, NFB], f32, name=f"m2{t}")
                    for t in range(TT)]
            q_p = [sbC.tile([128, NFB], f32, name=f"qp{t}")
                   for t in range(TT)]
            poolrow = sbC.tile([1, 4], f32)
            wacc = sbC.tile([128, 3 * FT], f32)   # |w| partial colsums

            if not ln_is_ones:
                Ln8k = sbC.tile([128, F], f32)

            # ============ P1: x per-token absmax -> scales ============
            for i in range(TT):
                xt = sbW.tile([128, D], f32, tag="wstage")
                nc.sync.dma_start(xt[:], xs_t.ap()[i * 128:(i + 1) * 128, :])
                am = sbCol.tile([128, 1], f32, tag="am")
                nc.vector.tensor_reduce(out=am[:], in_=xt[:], axis=AX.X,
                                        op=OP.max, apply_absolute_value=True)
                amc = sbCol.tile([128, 1], f32, tag="amc")
                nc.vector.tensor_scalar(out=amc[:], in0=am[:], scalar1=QEPS,
                                        scalar2=1.0 / 127.0, op0=OP.max,
                                        op1=OP.mult)
                # amc = absmax/127 ; deqb col ; reciprocal -> 127/absmax
                nc.vector.tensor_copy(deqb[:, i:i + 1], amc[:])
                rec = sbCol.tile([128, 1], f32, tag="rec")
                nc.vector.reciprocal(rec[:], amc[:])
                nc.scalar.dma_start(scd_dram[0:1, i * 128:(i + 1) * 128],
                                    rec[:])
            screc = sbR.tile([1, TK], f32, tag="screc")
            nc.scalar.dma_start(screc[:], scd_dram[:])
            psx = psB.tile([128, FBW], f32, tag="pb")
            nc.tensor.matmul(psx[:, 0:TK], ones1[:], screc[:], start=True,
                             stop=True)
            nc.vector.tensor_copy(Sb384[:], psx[:, 0:TK])

            # ============ P2: xq (int in bf16) ============
            for d in range(DT):
                xt = sbX.tile([128, TK], f32, tag="xT")
                nc.sync.dma_start(xt[:], xT_t.ap()[d * 128:(d + 1) * 128, :])
                prod = sbX.tile([128, TK], f32, tag="xprod")
                nc.vector.tensor_tensor(out=prod[:], in0=xt[:], in1=Sb384[:],
                                        op=OP.mult)
                nc.vector.tensor_scalar(out=xq[d][:], in0=prod[:],
                                        scalar1=MAGIC, scalar2=MAGIC,
                                        op0=OP.add, op1=OP.subtract)

            # ============ P3: weight |.| stats (full scans) ============
            # wg |.| colsums on Scalar (Abs + accum), wu/wd on Vector —
            # balances the two engines under the DMA roofline
            col = 0
            wcols = {}
            for name, ten in (("wg", wgf_t), ("wu", wuf_t), ("wd", wdT_t)):
                c0 = col
                for i in range(FT):
                    wt = sbW.tile([128, D], f32, tag="wstage")
                    nc.sync.dma_start(wt[:], ten.ap()[i * 128:(i + 1) * 128, :])
                    if name == "wg":
                        dump = sbQ2.tile([128, D], f32, tag="dump")
                        nc.scalar.activation(out=dump[:], in_=wt[:],
                                             func=AF.Abs,
                                             accum_out=wacc[:, col:col + 1])
                    else:
                        nc.vector.tensor_reduce(
                            out=wacc[:, col:col + 1], in_=wt[:], axis=AX.X,
                            op=OP.add, apply_absolute_value=True)
                    col += 1
                wcols[name] = (c0, col)

            m_col = {}
            s_col = {}
            for name in ("wg", "wu", "wd"):
                c0, c1 = wcols[name]
                csum = sbCol.tile([128, 1], f32, tag="csum")
                nc.vector.tensor_reduce(out=csum[:], in_=wacc[:, c0:c1],
                                        axis=AX.X, op=OP.add)
                tot = sbCol.tile([128, 1], f32, tag=f"tot_{name}")
                nc.gpsimd.partition_all_reduce(tot[:], csum[:], channels=128,
                                               reduce_op=RO.add)
                m = sbC.tile([128, 1], f32, name=f"m_{name}")
                nc.vector.tensor_scalar(out=m[:], in0=tot[:],
                                        scalar1=1.0 / (float(F) * D),
                                        scalar2=QEPS, op0=OP.mult, op1=OP.max)
                s = sbC.tile([128, 1], f32, name=f"s_{name}")
                nc.vector.reciprocal(s[:], m[:])
                m_col[name], s_col[name] = m, s
            nc.vector.tensor_scalar(out=DEQG[:], in0=deqb[:],
                                    scalar1=m_col["wg"][:, 0:1], scalar2=None,
                                    op0=OP.mult)
            nc.vector.tensor_scalar(out=DEQU[:], in0=deqb[:],
                                    scalar1=m_col["wu"][:, 0:1], scalar2=None,
                                    op0=OP.mult)

            def quant_chain(src, wdst, scol, total, width=1024):
                """src f32 -> ternary in wdst, in `width` chunks.
                wdst = clip(round(src * scol), -1, 1) via fp32 magic RNE."""
                for j in range(0, total, width):
                    w = min(width, total - j)
                    t1 = sbQ1.tile([128, width], f32, tag="t1")
                    nc.scalar.activation(out=t1[:, 0:w], in_=src[:, j:j + w],
                                         func=AF.Copy, scale=scol[:, 0:1],
                                         bias=MAGIC)
                    t1b = sbQ1.tile([128, width], f32, tag="t1b")
                    nc.vector.tensor_scalar(out=t1b[:, 0:w], in0=t1[:, 0:w],
                                            scalar1=MAGIC + 1.0, scalar2=MAGIC,
                                            op0=OP.min, op1=OP.subtract)
                    nc.vector.tensor_scalar(out=wdst[:, j:j + w],
                                            in0=t1b[:, 0:w], scalar1=-1.0,
                                            scalar2=None, op0=OP.max)

            # ============ P4: Wd quantize + colsum -> S ============
            # clip(round(z),-1,1) == (sign(z-1/2)+sign(z+1/2))/2 away from
            # the measure-zero tie points, so two Sign activations with
            # accum_out give 2*S[f] row sums with no Vector work; the /2 is
            # folded into invs2 below.
            Ssh = sbC.tile([128, FT], f32)
            for i in range(FT):
                wt = sbW.tile([128, D], f32, tag="wstage")
                nc.sync.dma_start(wt[:], wdT_t.ap()[i * 128:(i + 1) * 128, :])
                dump = sbQ2.tile([128, D], f32, tag="dump")
                colA = sbCol.tile([128, 1], f32, tag="colA")
                nc.scalar.activation(out=dump[:], in_=wt[:], func=AF.Sign,
                                     scale=s_col["wd"][:, 0:1],
                                     bias=neghalf[:], accum_out=colA[:])
                dump2 = sbQ2.tile([128, D], f32, tag="dump")
                colB = sbCol.tile([128, 1], f32, tag="colB")
                nc.scalar.activation(out=dump2[:], in_=wt[:], func=AF.Sign,
                                     scale=s_col["wd"][:, 0:1],
                                     bias=poshalf[:], accum_out=colB[:])
                nc.vector.tensor_tensor(out=Ssh[:, i:i + 1], in0=colA[:],
                                        in1=colB[:], op=OP.add)
            nc.scalar.dma_start(r128(srow_dram[:]), Ssh[:])
            for j in range(0, F, FBW):
                srow = sbR.tile([1, FBW], f32, tag="srow")
                nc.scalar.dma_start(srow[:], srow_dram[0:1, j:j + FBW])
                pb = psB.tile([128, FBW], f32, tag="pb")
                nc.tensor.matmul(pb[:], ones1[:], srow[:], start=True,
                                 stop=True)
                nc.vector.tensor_copy(S8k[:, j:j + FBW], pb[:])
            if not ln_is_ones:
                for j in range(0, F, FBW):
                    lrow = sbR.tile([1, FBW], f32, tag="srow")
                    nc.scalar.dma_start(lrow[:], lnw_t.ap()[0:1, j:j + FBW])
                    pb = psB.tile([128, FBW], f32, tag="pb")
                    nc.tensor.matmul(pb[:], ones1[:], lrow[:], start=True,
                                     stop=True)
                    nc.vector.tensor_copy(Ln8k[:, j:j + FBW], pb[:])

            # ============ P5: main pass — quantize wg/wu + matmuls ======
            for fb in range(NFB):
                ps = {}
                for mat, ten in (("wg", wgf_t), ("wu", wuf_t)):
                    slabs = []
                    for s in range(NSLAB):
                        wt = sbW.tile([128, NSLAB * FBW], f32, tag="wstage")
                        r0 = (fb * NSLAB + s) * 128
                        nc.sync.dma_start(wt[:], ten.ap()[r0:r0 + 128, :])
                        wq = sbWQ.tile([128, NSLAB * FBW], bf16, tag="wq")
                        quant_chain(wt, wq, s_col[mat], NSLAB * FBW)
                        slabs.append(wq)
                    for t in range(TT):
                        p = psM.tile([128, FBW], f32, tag=f"ps_{mat}{t}")
                        ps[(mat, t)] = p
                        for s in range(NSLAB):
                            for b in range(NSLAB):
                                dd = s * NSLAB + b
                                nc.tensor.matmul(
                                    p[:],
                                    xq[dd][:, t * 128:(t + 1) * 128],
                                    slabs[s][:, b * FBW:(b + 1) * FBW],
                                    start=(dd == 0), stop=(dd == DT - 1))
                for t in range(TT):
                    us = sbE.tile([128, FBW], f32, tag="us")
                    nc.scalar.activation(out=us[:], in_=ps[("wu", t)][:],
                                         func=AF.Copy,
                                         scale=DEQU[:, t:t + 1])
                    gsl = sbE.tile([128, FBW], f32, tag="gsl")
                    nc.scalar.activation(out=gsl[:], in_=ps[("wg", t)][:],
                                         func=AF.Silu,
                                         scale=DEQG[:, t:t + 1])
                    hsl = h_t[t][:, fb * FBW:(fb + 1) * FBW]
                    if ln_is_ones:
                        nc.vector.tensor_tensor(out=hsl, in0=gsl[:],
                                                in1=us[:], op=OP.mult)
                        hsq = sbE.tile([128, FBW], f32, tag="hsq")
                        nc.scalar.activation(
                            out=hsq[:], in_=hsl, func=AF.Square,
                            accum_out=ssq_p[t][:, fb:fb + 1])
                        nc.vector.tensor_reduce(
                            out=m2_p[t][:, fb:fb + 1], in_=hsq[:], axis=AX.X,
                            op=OP.max)
                    else:
                        hraw = sbE.tile([128, FBW], f32, tag="hraw")
                        nc.vector.tensor_tensor(out=hraw[:], in0=gsl[:],
                                                in1=us[:], op=OP.mult)
                        hsq = sbE.tile([128, FBW], f32, tag="hsq")
                        nc.scalar.activation(
                            out=hsq[:], in_=hraw[:], func=AF.Square,
                            accum_out=ssq_p[t][:, fb:fb + 1])
                        nc.vector.tensor_tensor(
                            out=hsl, in0=hraw[:],
                            in1=Ln8k[:, fb * FBW:(fb + 1) * FBW], op=OP.mult)
                        h2q = sbE.tile([128, FBW], f32, tag="h2q")
                        nc.scalar.activation(out=h2q[:], in_=hsl,
                                             func=AF.Square)
                        nc.vector.tensor_reduce(
                            out=m2_p[t][:, fb:fb + 1], in_=h2q[:], axis=AX.X,
                            op=OP.max)

            # ====== P6: rmsnorm + requant + down-proj pooled, per tile ======
            for t in range(TT):
                ssq = sbCol.tile([128, 1], f32, tag="st_a")
                nc.vector.tensor_reduce(out=ssq[:], in_=ssq_p[t][:],
                                        axis=AX.X, op=OP.add)
                m2 = sbCol.tile([128, 1], f32, tag="st_b")
                nc.vector.tensor_reduce(out=m2[:], in_=m2_p[t][:],
                                        axis=AX.X, op=OP.max)
                v = sbCol.tile([128, 1], f32, tag="st_c")
                nc.vector.tensor_scalar(out=v[:], in0=ssq[:],
                                        scalar1=1.0 / F, scalar2=EPS,
                                        op0=OP.mult, op1=OP.add)
                sv = sbCol.tile([128, 1], f32, tag="st_d")
                nc.scalar.activation(out=sv[:], in_=v[:], func=AF.Sqrt)
                rs = sbCol.tile([128, 1], f32, tag="st_e")
                nc.vector.reciprocal(rs[:], sv[:])
                gmax = sbCol.tile([128, 1], f32, tag="st_f")
                nc.scalar.activation(out=gmax[:], in_=m2[:], func=AF.Sqrt)
                rg = sbCol.tile([128, 1], f32, tag="st_g")
                nc.vector.tensor_tensor(out=rg[:], in0=rs[:], in1=gmax[:],
                                        op=OP.mult)
                y2 = sbCol.tile([128, 1], f32, tag="st_h")
                nc.vector.tensor_scalar(out=y2[:], in0=rg[:], scalar1=QEPS,
                                        scalar2=None, op0=OP.max)
                invs2 = sbCol.tile([128, 1], f32, tag="st_i")
                # extra 1/2 compensates Ssh holding 2*S (Sign-pair quant)
                nc.vector.tensor_scalar(
                    out=invs2[:], in0=y2[:], scalar1=m_col["wd"][:, 0:1],
                    scalar2=1.0 / (2.0 * 127.0 * float(H) * D), op0=OP.mult,
                    op1=OP.mult)
                r2 = sbCol.tile([128, 1], f32, tag="st_j")
                nc.vector.reciprocal(r2[:], y2[:])
                al = sbCol.tile([128, 1], f32, tag="st_k")
                nc.vector.tensor_tensor(out=al[:], in0=r2[:], in1=rs[:],
                                        op=OP.mult)
                alpha = sbCol.tile([128, 1], f32, tag="st_l")
                nc.vector.tensor_scalar(out=alpha[:], in0=al[:],
                                        scalar1=127.0, scalar2=None,
                                        op0=OP.mult)
                for j in range(NFB):
                    hsl = h_t[t][:, j * FBW:(j + 1) * FBW]
                    w1 = sbE.tile([128, FBW], f32, tag="us")
                    nc.vector.tensor_scalar(out=w1[:], in0=hsl,
                                            scalar1=alpha[:, 0:1],
                                            scalar2=MAGIC, op0=OP.mult,
                                            op1=OP.add)
                    hq = sbE.tile([128, FBW], f32, tag="gsl")
                    nc.scalar.activation(out=hq[:], in_=w1[:],
                                         func=AF.Identity, bias=negmagic[:])
                    junk = sbE.tile([128, FBW], f32, tag="hsq")
                    nc.vector.tensor_tensor(out=junk[:], in0=hq[:],
                                            in1=S8k[:, j * FBW:(j + 1) * FBW],
                                            op=OP.mult)
                    nc.vector.tensor_reduce(out=q_p[t][:, j:j + 1],
                                            in_=junk[:], axis=AX.X, op=OP.add)
                qsum = sbCol.tile([128, 1], f32, tag="qsum")
                nc.vector.tensor_reduce(out=qsum[:], in_=q_p[t][:],
                                        axis=AX.X, op=OP.add)
                qd = sbCol.tile([128, 1], f32, tag="qd")
                nc.vector.tensor_scalar(out=qd[:], in0=qsum[:],
                                        scalar1=invs2[:, 0:1], scalar2=None,
                                        op0=OP.mult)
                qall = sbCol.tile([128, 1], f32, tag="qall")
                nc.gpsimd.partition_all_reduce(qall[:], qd[:], channels=128,
                                               reduce_op=RO.add)
                nc.vector.tensor_copy(poolrow[0:1, t:t + 1], qall[0:1, :])

            # ============ P7: classifier row k ============
            nc.scalar.dma_start(pl_dram[:], poolrow[:])
            pool3 = sbC.tile([C, 1], f32)
            nc.scalar.dma_start(
                pool3[:],
                pl_dram[0:1, 0:C].rearrange("o (p i) -> (o p) i", p=C))
            out_sb = sbC.tile([1, NCLS], f32)
            for j in range(0, NCLS, FBW):
                w = min(FBW, NCLS - j)
                pcls = psB.tile([128, FBW], f32, tag="pb")
                nc.tensor.matmul(pcls[0:1, 0:w], pool3[:], clsW_sb[:, j:j + w],
                                 start=True, stop=False)
                nc.tensor.matmul(pcls[0:1, 0:w], ones1[0:1, 0:1],
                                 clsb_sb[:, j:j + w], start=False, stop=True)
                nc.vector.tensor_copy(out_sb[:, j:j + w], pcls[0:1, 0:w])
            nc.scalar.dma_start(out_t.ap(), out_sb[:])

    nc.compile()
    meta = dict(B=B, C=C, H=H, D=D, F=F, NCLS=NCLS, NCORES=NCORES,
                TK=TK, FBW=FBW, NFB=NFB)
    return nc, meta


def make_in_maps(x, Wg, Wu, Wd, ln_w, cls_W, cls_b, meta):
    """Host-side sharding: slices/transposes/reshapes only, no arithmetic."""
    B, C, H, D = meta["B"], meta["C"], meta["H"], meta["D"]
    F, NCLS, NCORES = meta["F"], meta["NCLS"], meta["NCORES"]
    TK, FBW, NFB = meta["TK"], meta["FBW"], meta["NFB"]
    xf = np.ascontiguousarray(np.asarray(x, np.float32).reshape(B * C * H, D))
    # pre-tiled layouts of Wg.T / Wu.T: tile (fb, s) is [128, 4*FBW] with
    # partition p = d-row s*512 + b*128 + p for col block b
    NSLAB = 4
    def pretile(W):
        WT = np.asarray(W, np.float32).T            # [D, F]
        X = WT.reshape(NSLAB, NSLAB, 128, NFB, FBW)  # (s, b, p, fb, c)
        return np.ascontiguousarray(
            X.transpose(3, 0, 2, 1, 4).reshape(F, D))
    wgf = pretile(Wg)
    wuf = pretile(Wu)
    wdT = np.ascontiguousarray(np.asarray(Wd, np.float32).T)
    clsWT = np.ascontiguousarray(np.asarray(cls_W, np.float32).T)
    clsb2 = np.ascontiguousarray(
        np.asarray(cls_b, np.float32).reshape(1, NCLS))
    ln_ones = bool(np.all(np.asarray(ln_w) == 1.0))
    maps = []
    for k in range(NCORES):
        m = {
            "xs": np.ascontiguousarray(xf[k * TK:(k + 1) * TK]),
            "xT": np.ascontiguousarray(xf[k * TK:(k + 1) * TK].T),
            "wgf": wgf,
            "wuf": wuf,
            "wdT": wdT,
            "clsWT": clsWT,
            "clsb": clsb2,
        }
        if not ln_ones:
            m["lnw"] = np.ascontiguousarray(
                np.asarray(ln_w, np.float32).reshape(1, F))
        maps.append(m)
    return maps


_CACHE = {}


def kernel(x, Wg, Wu, Wd, ln_w, cls_W, cls_b):
    """Takes FULL inputs, runs the 8-core DP Bass kernel, returns [B, NCLS]."""
    from concourse import bass_utils

    x = np.asarray(x, np.float32)
    B, C, H, D = x.shape
    F = int(np.asarray(Wg).shape[0])
    NCLS = int(np.asarray(cls_W).shape[0])
    ln_ones = bool(np.all(np.asarray(ln_w) == 1.0))
    key = (B, C, H, D, F, NCLS, ln_ones)
    if key not in _CACHE:
        _CACHE[key] = build(B=B, C=C, H=H, D=D, F=F, NCLS=NCLS, NCORES=8,
                            ln_is_ones=ln_ones)
    nc, meta = _CACHE[key]
    in_maps = make_in_maps(x, Wg, Wu, Wd, ln_w, cls_W, cls_b, meta)
    res = bass_utils.run_bass_kernel_spmd(nc, in_maps, core_ids=list(range(8)))
    return np.concatenate(
        [np.asarray(res.results[k]["out"], np.float32) for k in range(8)],
        axis=0)


# revision 26
# speedup vs baseline: 1.1147x; 1.0724x over previous
"""BitNet SwiGLU MLP kernel for Trainium2 — zero-collective data-parallel
over 8 NeuronCores.

Sharding: pure data-parallel over the batch dim. Core k handles batch row
b=k (C*H = 384 tokens = 3 token tiles) and reads the FULL weights. Every
global quantity is computed core-locally, so the kernel has NO collectives
and no cross-core sync points — each core's NEFF runs independently of
launch skew:
- per-token activation absmax: from the core's own 384 token rows.
- global weight scales 1/mean|W|: each core scans the full Wg/Wu/Wd
  (it must stream them for the matmuls anyway; one extra stats pass).
- RMSNorm + requant stats over the full intermediate F: rows are complete
  per core (no F-sharding), so stats are local.
- down-proj + mean-pool collapses algebraically (as in the TP variant):
  mean_{h,d}(hq @ Wdq.T) = 1/(H*D) * sum_f (sum_h hq[t,f]) * S[f] with
  S[f] = sum_d Wdq[d,f], so Wd is consumed as a quantize+colsum scan only.
- classifier: core k's pooled row [C] is complete locally -> it computes
  output row k. Host assembles rows (pure gather, no arithmetic).

bit_linear is an exact integer matmul in bf16 (ints |.|<=127 and ternary
weights are exact in bf16; fp32 PSUM accumulation), with fp32
magic-number RNE rounding for the quantizers. h is kept resident in SBUF
as fp16 (never spilled to DRAM). DMA ~410MB/core of weight traffic at
~360GB/s is the roofline; PE/Vector/Scalar all have slack under it.
"""
import numpy as np

MAGIC = 12582912.0  # 1.5 * 2^23, fp32 RNE magic
EPS = 1e-6
QEPS = 1e-5


def build(B=8, C=3, H=128, D=2048, F=8192, NCLS=1000, NCORES=8,
          ln_is_ones=True):
    """Build + compile the per-core Bass program. Returns (nc, meta)."""
    import concourse.bacc as bacc
    import concourse.tile as tile
    from concourse import mybir
    from concourse import bass_isa

    f32 = mybir.dt.float32
    bf16 = mybir.dt.bfloat16
    fp16 = mybir.dt.float16
    AX = mybir.AxisListType
    OP = mybir.AluOpType
    AF = mybir.ActivationFunctionType
    RO = bass_isa.ReduceOp

    assert H == 128 and B == NCORES
    TK = C * H                 # tokens per core (384)
    TT = C                     # token tiles per core (3)
    DT = D // 128              # contraction tiles (16)
    FBW = 512                  # f-block width (psum bank per (mat, t))
    NFB = F // FBW             # 16 f-blocks
    SLAB = 512                 # weight rows (d) per main-pass DMA
    NSLAB = D // SLAB          # 4 slabs per f-block
    FT = F // 128              # 64 wd row tiles

    nc = bacc.Bacc("TRN2", target_bir_lowering=False, debug=False,
                   num_devices=NCORES)

    xs_t = nc.dram_tensor("xs", [TK, D], f32, kind="ExternalInput")
    xT_t = nc.dram_tensor("xT", [D, TK], f32, kind="ExternalInput")
    # pre-tiled on host: row-block (fb*NSLAB+s)*128+p, col b*FBW+c holds
    # Wg.T[s*SLAB + b*128 + p, fb*FBW + c] -> every DMA below is a plain
    # [128, 2048] slice with 8KB contiguous per partition row
    wgf_t = nc.dram_tensor("wgf", [F, D], f32, kind="ExternalInput")
    wuf_t = nc.dram_tensor("wuf", [F, D], f32, kind="ExternalInput")
    wdT_t = nc.dram_tensor("wdT", [F, D], f32, kind="ExternalInput")
    if not ln_is_ones:
        lnw_t = nc.dram_tensor("lnw", [1, F], f32, kind="ExternalInput")
    clsWT_t = nc.dram_tensor("clsWT", [C, NCLS], f32, kind="ExternalInput")
    clsb_t = nc.dram_tensor("clsb", [1, NCLS], f32, kind="ExternalInput")
    out_t = nc.dram_tensor("out", [1, NCLS], f32, kind="ExternalOutput")

    def r128(ap):
        # [1, n*128] dram view -> [128, n] (partition = fast axis)
        return ap.rearrange("o (i p) -> (o p) i", p=128)

    with tile.TileContext(nc) as tc:
        import contextlib
        with contextlib.ExitStack() as st:
            dram = st.enter_context(tc.tile_pool(name="dram", bufs=1,
                                                 space="DRAM"))
            sbC = st.enter_context(tc.tile_pool(name="sbC", bufs=1))
            sbW = st.enter_context(tc.tile_pool(name="sbW", bufs=4))
            sbQ1 = st.enter_context(tc.tile_pool(name="sbQ1", bufs=2))
            sbQ2 = st.enter_context(tc.tile_pool(name="sbQ2", bufs=1))
            sbWQ = st.enter_context(tc.tile_pool(name="sbWQ", bufs=5))
            sbE = st.enter_context(tc.tile_pool(name="sbE", bufs=2))
            sbR = st.enter_context(tc.tile_pool(name="sbR", bufs=2))
            sbX = st.enter_context(tc.tile_pool(name="sbX", bufs=3))
            sbCol = st.enter_context(tc.tile_pool(name="sbCol", bufs=3))
            psM = st.enter_context(tc.tile_pool(name="psM", bufs=1,
                                                space="PSUM"))
            psB = st.enter_context(tc.tile_pool(name="psB", bufs=1,
                                                space="PSUM"))

            srow_dram = dram.tile([1, F], f32)
            scd_dram = dram.tile([1, TK], f32)
            pl_dram = dram.tile([1, 4], f32)

            ones1 = sbC.tile([1, 128], f32)
            nc.vector.memset(ones1[:], 1.0)
            negmagic = sbC.tile([128, 1], f32)
            nc.vector.memset(negmagic[:], -MAGIC)
            neghalf = sbC.tile([128, 1], f32)
            nc.vector.memset(neghalf[:], -0.5)
            poshalf = sbC.tile([128, 1], f32)
            nc.vector.memset(poshalf[:], 0.5)
            clsW_sb = sbC.tile([C, NCLS], f32)
            nc.scalar.dma_start(clsW_sb[:], clsWT_t.ap())
            clsb_sb = sbC.tile([1, NCLS], f32)
            nc.scalar.dma_start(clsb_sb[:], clsb_t.ap())

            # persistent state
            h_t = [sbC.tile([128, F], fp16, name=f"h{t}") for t in range(TT)]
            xq = [sbC.tile([128, TK], bf16, name=f"xq{d}") for d in range(DT)]
            S8k = sbC.tile([128, F], bf16)        # colsum(Wdq) row-bcast
            Sb384 = sbC.tile([128, TK], f32)      # 127/absmax per token
            deqb = sbC.tile([128, TT], f32)       # absmax/127 per token
            DEQG = sbC.tile([128, TT], f32)
            DEQU = sbC.tile([128, TT], f32)
            ssq_p = [sbC.tile([128, NFB], f32, name=f"ssq{t}")
                     for t in range(TT)]
            m2_p = [sbC.tile([128, NFB], f32, name=f"m2{t}")
                    for t in range(TT)]
            q_p = [sbC.tile([128, NFB], f32, name=f"qp{t}")
                   for t in range(TT)]
            poolrow = sbC.tile([1, 4], f32)
            wacc = sbC.tile([128, 3 * FT], f32)   # |w| partial colsums

            if not ln_is_ones:
                Ln8k = sbC.tile([128, F], f32)

            # ============ P1: x per-token absmax -> scales ============
            for i in range(TT):
                xt = sbW.tile([128, D], f32, tag="wstage")
                nc.sync.dma_start(xt[:], xs_t.ap()[i * 128:(i + 1) * 128, :])
                am = sbCol.tile([128, 1], f32, tag="am")
                nc.vector.tensor_reduce(out=am[:], in_=xt[:], axis=AX.X,
                                        op=OP.max, apply_absolute_value=True)
                amc = sbCol.tile([128, 1], f32, tag="amc")
                nc.vector.tensor_scalar(out=amc[:], in0=am[:], scalar1=QEPS,
                                        scalar2=1.0 / 127.0, op0=OP.max,
                                        op1=OP.mult)
                # amc = absmax/127 ; deqb col ; reciprocal -> 127/absmax
                nc.vector.tensor_copy(deqb[:, i:i + 1], amc[:])
                rec = sbCol.tile([128, 1], f32, tag="rec")
                nc.vector.reciprocal(rec[:], amc[:])
                nc.scalar.dma_start(scd_dram[0:1, i * 128:(i + 1) * 128],
                                    rec[:])
            screc = sbR.tile([1, TK], f32, tag="screc")
            nc.scalar.dma_start(screc[:], scd_dram[:])
            psx = psB.tile([128, FBW], f32, tag="pb")
            nc.tensor.matmul(psx[:, 0:TK], ones1[:], screc[:], start=True,
                             stop=True)
            nc.vector.tensor_copy(Sb384[:], psx[:, 0:TK])

            # ============ P2: xq (int in bf16) ============
            for d in range(DT):
                xt = sbX.tile([128, TK], f32, tag="xT")
                nc.sync.dma_start(xt[:], xT_t.ap()[d * 128:(d + 1) * 128, :])
                prod = sbX.tile([128, TK], f32, tag="xprod")
                nc.vector.tensor_tensor(out=prod[:], in0=xt[:], in1=Sb384[:],
                                        op=OP.mult)
                nc.vector.tensor_scalar(out=xq[d][:], in0=prod[:],
                                        scalar1=MAGIC, scalar2=MAGIC,
                                        op0=OP.add, op1=OP.subtract)

            # ============ P3: weight |.| stats (full scans) ============
            m_col = {}
            s_col = {}

            def weight_scalars(name, c0, c1):
                csum = sbCol.tile([128, 1], f32, tag="csum")
                nc.vector.tensor_reduce(out=csum[:], in_=wacc[:, c0:c1],
                                        axis=AX.X, op=OP.add)
                tot = sbCol.tile([128, 1], f32, tag=f"tot_{name}")
                nc.gpsimd.partition_all_reduce(tot[:], csum[:], channels=128,
                                               reduce_op=RO.add)
                m = sbC.tile([128, 1], f32, name=f"m_{name}")
                nc.vector.tensor_scalar(out=m[:], in0=tot[:],
                                        scalar1=1.0 / (float(F) * D),
                                        scalar2=QEPS, op0=OP.mult, op1=OP.max)
                s = sbC.tile([128, 1], f32, name=f"s_{name}")
                nc.vector.reciprocal(s[:], m[:])
                m_col[name], s_col[name] = m, s

            # P3a: wd |.| stats first (Vector), so the wd quant pass can
            # interleave with wg/wu stats below
            for i in range(FT):
                wt = sbW.tile([128, D], f32, tag="wstage")
                nc.sync.dma_start(wt[:], wdT_t.ap()[i * 128:(i + 1) * 128, :])
                nc.vector.tensor_reduce(
                    out=wacc[:, i:i + 1], in_=wt[:], axis=AX.X,
                    op=OP.add, apply_absolute_value=True)
            weight_scalars("wd", 0, FT)

            # P3b: interleave wg/wu stats (Vector) with the wd Sign-pair
            # quant+colsum (Scalar) so neither engine starves the DMA queue.
            # clip(round(z),-1,1) == (sign(z-1/2)+sign(z+1/2))/2 away from
            # the measure-zero tie points; accum_out gives the d-colsums of
            # 2*S[f] with no Vector work (the /2 is folded into invs2).
            Ssh = sbC.tile([128, FT], f32)
            for i in range(FT):
                for j, ten in ((1, wgf_t), (2, wuf_t)):
                    wt = sbW.tile([128, D], f32, tag="wstage")
                    nc.sync.dma_start(wt[:],
                                      ten.ap()[i * 128:(i + 1) * 128, :])
                    nc.vector.tensor_reduce(
                        out=wacc[:, j * FT + i:j * FT + i + 1], in_=wt[:],
                        axis=AX.X, op=OP.add, apply_absolute_value=True)
                wt = sbW.tile([128, D], f32, tag="wstage")
                nc.sync.dma_start(wt[:], wdT_t.ap()[i * 128:(i + 1) * 128, :])
                dump = sbQ2.tile([128, D], f32, tag="dump")
                colA = sbCol.tile([128, 1], f32, tag="colA")
                nc.scalar.activation(out=dump[:], in_=wt[:], func=AF.Sign,
                                     scale=s_col["wd"][:, 0:1],
                                     bias=neghalf[:], accum_out=colA[:])
                dump2 = sbQ2.tile([128, D], f32, tag="dump")
                colB = sbCol.tile([128, 1], f32, tag="colB")
                nc.scalar.activation(out=dump2[:], in_=wt[:], func=AF.Sign,
                                     scale=s_col["wd"][:, 0:1],
                                     bias=poshalf[:], accum_out=colB[:])
                nc.vector.tensor_tensor(out=Ssh[:, i:i + 1], in0=colA[:],
                                        in1=colB[:], op=OP.add)
            weight_scalars("wg", FT, 2 * FT)
            weight_scalars("wu", 2 * FT, 3 * FT)
            nc.vector.tensor_scalar(out=DEQG[:], in0=deqb[:],
                                    scalar1=m_col["wg"][:, 0:1], scalar2=None,
                                    op0=OP.mult)
            nc.vector.tensor_scalar(out=DEQU[:], in0=deqb[:],
                                    scalar1=m_col["wu"][:, 0:1], scalar2=None,
                                    op0=OP.mult)

            def quant_chain(src, wdst, scol, total, width=1024):
                """src f32 -> ternary in wdst, in `width` chunks.
                wdst = clip(round(src * scol), -1, 1) via fp32 magic RNE."""
                for j in range(0, total, width):
                    w = min(width, total - j)
                    t1 = sbQ1.tile([128, width], f32, tag="t1")
                    nc.scalar.activation(out=t1[:, 0:w], in_=src[:, j:j + w],
                                         func=AF.Copy, scale=scol[:, 0:1],
                                         bias=MAGIC)
                    t1b = sbQ1.tile([128, width], f32, tag="t1b")
                    nc.vector.tensor_scalar(out=t1b[:, 0:w], in0=t1[:, 0:w],
                                            scalar1=MAGIC + 1.0, scalar2=MAGIC,
                                            op0=OP.min, op1=OP.subtract)
                    nc.vector.tensor_scalar(out=wdst[:, j:j + w],
                                            in0=t1b[:, 0:w], scalar1=-1.0,
                                            scalar2=None, op0=OP.max)

            # ============ P4: broadcast S across partitions ============
            nc.scalar.dma_start(r128(srow_dram[:]), Ssh[:])
            for j in range(0, F, FBW):
                srow = sbR.tile([1, FBW], f32, tag="srow")
                nc.scalar.dma_start(srow[:], srow_dram[0:1, j:j + FBW])
                pb = psB.tile([128, FBW], f32, tag="pb")
                nc.tensor.matmul(pb[:], ones1[:], srow[:], start=True,
                                 stop=True)
                nc.vector.tensor_copy(S8k[:, j:j + FBW], pb[:])
            if not ln_is_ones:
                for j in range(0, F, FBW):
                    lrow = sbR.tile([1, FBW], f32, tag="srow")
                    nc.scalar.dma_start(lrow[:], lnw_t.ap()[0:1, j:j + FBW])
                    pb = psB.tile([128, FBW], f32, tag="pb")
                    nc.tensor.matmul(pb[:], ones1[:], lrow[:], start=True,
                                     stop=True)
                    nc.vector.tensor_copy(Ln8k[:, j:j + FBW], pb[:])

            # ============ P5: main pass — quantize wg/wu + matmuls ======
            for fb in range(NFB):
                ps = {}
                for mat, ten in (("wg", wgf_t), ("wu", wuf_t)):
                    slabs = []
                    for s in range(NSLAB):
                        wt = sbW.tile([128, NSLAB * FBW], f32, tag="wstage")
                        r0 = (fb * NSLAB + s) * 128
                        nc.sync.dma_start(wt[:], ten.ap()[r0:r0 + 128, :])
                        wq = sbWQ.tile([128, NSLAB * FBW], bf16, tag="wq")
                        quant_chain(wt, wq, s_col[mat], NSLAB * FBW)
                        slabs.append(wq)
                    for t in range(TT):
                        p = psM.tile([128, FBW], f32, tag=f"ps_{mat}{t}")
                        ps[(mat, t)] = p
                        for s in range(NSLAB):
                            for b in range(NSLAB):
                                dd = s * NSLAB + b
                                nc.tensor.matmul(
                                    p[:],
                                    xq[dd][:, t * 128:(t + 1) * 128],
                                    slabs[s][:, b * FBW:(b + 1) * FBW],
                                    start=(dd == 0), stop=(dd == DT - 1))
                for t in range(TT):
                    us = sbE.tile([128, FBW], f32, tag="us")
                    nc.scalar.activation(out=us[:], in_=ps[("wu", t)][:],
                                         func=AF.Copy,
                                         scale=DEQU[:, t:t + 1])
                    gsl = sbE.tile([128, FBW], f32, tag="gsl")
                    nc.scalar.activation(out=gsl[:], in_=ps[("wg", t)][:],
                                         func=AF.Silu,
                                         scale=DEQG[:, t:t + 1])
                    hsl = h_t[t][:, fb * FBW:(fb + 1) * FBW]
                    if ln_is_ones:
                        nc.vector.tensor_tensor(out=hsl, in0=gsl[:],
                                                in1=us[:], op=OP.mult)
                        hsq = sbE.tile([128, FBW], f32, tag="hsq")
                        nc.scalar.activation(
                            out=hsq[:], in_=hsl, func=AF.Square,
                            accum_out=ssq_p[t][:, fb:fb + 1])
                        nc.vector.tensor_reduce(
                            out=m2_p[t][:, fb:fb + 1], in_=hsq[:], axis=AX.X,
                            op=OP.max)
                    else:
                        hraw = sbE.tile([128, FBW], f32, tag="hraw")
                        nc.vector.tensor_tensor(out=hraw[:], in0=gsl[:],
                                                in1=us[:], op=OP.mult)
                        hsq = sbE.tile([128, FBW], f32, tag="hsq")
                        nc.scalar.activation(
                            out=hsq[:], in_=hraw[:], func=AF.Square,
                            accum_out=ssq_p[t][:, fb:fb + 1])
                        nc.vector.tensor_tensor(
                            out=hsl, in0=hraw[:],
                            in1=Ln8k[:, fb * FBW:(fb + 1) * FBW], op=OP.mult)
                        h2q = sbE.tile([128, FBW], f32, tag="h2q")
                        nc.scalar.activation(out=h2q[:], in_=hsl,
                                             func=AF.Square)
                        nc.vector.tensor_reduce(
                            out=m2_p[t][:, fb:fb + 1], in_=h2q[:], axis=AX.X,
                            op=OP.max)

            # ====== P6: rmsnorm + requant + down-proj pooled, per tile ======
            for t in range(TT):
                ssq = sbCol.tile([128, 1], f32, tag="st_a")
                nc.vector.tensor_reduce(out=ssq[:], in_=ssq_p[t][:],
                                        axis=AX.X, op=OP.add)
                m2 = sbCol.tile([128, 1], f32, tag="st_b")
                nc.vector.tensor_reduce(out=m2[:], in_=m2_p[t][:],
                                        axis=AX.X, op=OP.max)
                v = sbCol.tile([128, 1], f32, tag="st_c")
                nc.vector.tensor_scalar(out=v[:], in0=ssq[:],
                                        scalar1=1.0 / F, scalar2=EPS,
                                        op0=OP.mult, op1=OP.add)
                sv = sbCol.tile([128, 1], f32, tag="st_d")
                nc.scalar.activation(out=sv[:], in_=v[:], func=AF.Sqrt)
                rs = sbCol.tile([128, 1], f32, tag="st_e")
                nc.vector.reciprocal(rs[:], sv[:])
                gmax = sbCol.tile([128, 1], f32, tag="st_f")
                nc.scalar.activation(out=gmax[:], in_=m2[:], func=AF.Sqrt)
                rg = sbCol.tile([128, 1], f32, tag="st_g")
                nc.vector.tensor_tensor(out=rg[:], in0=rs[:], in1=gmax[:],
                                        op=OP.mult)
                y2 = sbCol.tile([128, 1], f32, tag="st_h")
                nc.vector.tensor_scalar(out=y2[:], in0=rg[:], scalar1=QEPS,
                                        scalar2=None, op0=OP.max)
                invs2 = sbCol.tile([128, 1], f32, tag="st_i")
                # extra 1/2 compensates Ssh holding 2*S (Sign-pair quant)
                nc.vector.tensor_scalar(
                    out=invs2[:], in0=y2[:], scalar1=m_col["wd"][:, 0:1],
                    scalar2=1.0 / (2.0 * 127.0 * float(H) * D), op0=OP.mult,
                    op1=OP.mult)
                r2 = sbCol.tile([128, 1], f32, tag="st_j")
                nc.vector.reciprocal(r2[:], y2[:])
                al = sbCol.tile([128, 1], f32, tag="st_k")
                nc.vector.tensor_tensor(out=al[:], in0=r2[:], in1=rs[:],
                                        op=OP.mult)
                alpha = sbCol.tile([128, 1], f32, tag="st_l")
                nc.vector.tensor_scalar(out=alpha[:], in0=al[:],
                                        scalar1=127.0, scalar2=None,
                                        op0=OP.mult)
                for j in range(NFB):
                    hsl = h_t[t][:, j * FBW:(j + 1) * FBW]
                    w1 = sbE.tile([128, FBW], f32, tag="us")
                    nc.scalar.activation(out=w1[:], in_=hsl, func=AF.Copy,
                                         scale=alpha[:, 0:1], bias=MAGIC)
                    hq = sbE.tile([128, FBW], f32, tag="gsl")
                    nc.scalar.activation(out=hq[:], in_=w1[:],
                                         func=AF.Identity, bias=negmagic[:])
                    junk = sbE.tile([128, FBW], f32, tag="hsq")
                    nc.vector.tensor_tensor(out=junk[:], in0=hq[:],
                                            in1=S8k[:, j * FBW:(j + 1) * FBW],
                                            op=OP.mult)
                    nc.vector.tensor_reduce(out=q_p[t][:, j:j + 1],
                                            in_=junk[:], axis=AX.X, op=OP.add)
                qsum = sbCol.tile([128, 1], f32, tag="qsum")
                nc.vector.tensor_reduce(out=qsum[:], in_=q_p[t][:],
                                        axis=AX.X, op=OP.add)
                qd = sbCol.tile([128, 1], f32, tag="qd")
                nc.vector.tensor_scalar(out=qd[:], in0=qsum[:],
                                        scalar1=invs2[:, 0:1], scalar2=None,
                                        op0=OP.mult)
                qall = sbCol.tile([128, 1], f32, tag="qall")
                nc.gpsimd.partition_all_reduce(qall[:], qd[:], channels=128,
                                               reduce_op=RO.add)
                nc.vector.tensor_copy(poolrow[0:1, t:t + 1], qall[0:1, :])

            # ============ P7: classifier row k ============
            nc.scalar.dma_start(pl_dram[:], poolrow[:])
            pool3 = sbC.tile([C, 1], f32)
            nc.scalar.dma_start(
                pool3[:],
                pl_dram[0:1, 0:C].rearrange("o (p i) -> (o p) i", p=C))
            out_sb = sbC.tile([1, NCLS], f32)
            for j in range(0, NCLS, FBW):
                w = min(FBW, NCLS - j)
                pcls = psB.tile([128, FBW], f32, tag="pb")
                nc.tensor.matmul(pcls[0:1, 0:w], pool3[:], clsW_sb[:, j:j + w],
                                 start=True, stop=False)
                nc.tensor.matmul(pcls[0:1, 0:w], ones1[0:1, 0:1],
                                 clsb_sb[:, j:j + w], start=False, stop=True)
                nc.vector.tensor_copy(out_sb[:, j:j + w], pcls[0:1, 0:w])
            nc.scalar.dma_start(out_t.ap(), out_sb[:])

    nc.compile()
    meta = dict(B=B, C=C, H=H, D=D, F=F, NCLS=NCLS, NCORES=NCORES,
                TK=TK, FBW=FBW, NFB=NFB)
    return nc, meta


def make_in_maps(x, Wg, Wu, Wd, ln_w, cls_W, cls_b, meta):
    """Host-side sharding: slices/transposes/reshapes only, no arithmetic."""
    B, C, H, D = meta["B"], meta["C"], meta["H"], meta["D"]
    F, NCLS, NCORES = meta["F"], meta["NCLS"], meta["NCORES"]
    TK, FBW, NFB = meta["TK"], meta["FBW"], meta["NFB"]
    xf = np.ascontiguousarray(np.asarray(x, np.float32).reshape(B * C * H, D))
    # pre-tiled layouts of Wg.T / Wu.T: tile (fb, s) is [128, 4*FBW] with
    # partition p = d-row s*512 + b*128 + p for col block b
    NSLAB = 4
    def pretile(W):
        WT = np.asarray(W, np.float32).T            # [D, F]
        X = WT.reshape(NSLAB, NSLAB, 128, NFB, FBW)  # (s, b, p, fb, c)
        return np.ascontiguousarray(
            X.transpose(3, 0, 2, 1, 4).reshape(F, D))
    wgf = pretile(Wg)
    wuf = pretile(Wu)
    wdT = np.ascontiguousarray(np.asarray(Wd, np.float32).T)
    clsWT = np.ascontiguousarray(np.asarray(cls_W, np.float32).T)
    clsb2 = np.ascontiguousarray(
        np.asarray(cls_b, np.float32).reshape(1, NCLS))
    ln_ones = bool(np.all(np.asarray(ln_w) == 1.0))
    maps = []
    for k in range(NCORES):
        m = {
            "xs": np.ascontiguousarray(xf[k * TK:(k + 1) * TK]),
            "xT": np.ascontiguousarray(xf[k * TK:(k + 1) * TK].T),
            "wgf": wgf,
            "wuf": wuf,
            "wdT": wdT,
            "clsWT": clsWT,
            "clsb": clsb2,
        }
        if not ln_ones:
            m["lnw"] = np.ascontiguousarray(
                np.asarray(ln_w, np.float32).reshape(1, F))
        maps.append(m)
    return maps


_CACHE = {}


def kernel(x, Wg, Wu, Wd, ln_w, cls_W, cls_b):
    """Takes FULL inputs, runs the 8-core DP Bass kernel, returns [B, NCLS]."""
    from concourse import bass_utils

    x = np.asarray(x, np.float32)
    B, C, H, D = x.shape
    F = int(np.asarray(Wg).shape[0])
    NCLS = int(np.asarray(cls_W).shape[0])
    ln_ones = bool(np.all(np.asarray(ln_w) == 1.0))
    key = (B, C, H, D, F, NCLS, ln_ones)
    if key not in _CACHE:
        _CACHE[key] = build(B=B, C=C, H=H, D=D, F=F, NCLS=NCLS, NCORES=8,
                            ln_is_ones=ln_ones)
    nc, meta = _CACHE[key]
    in_maps = make_in_maps(x, Wg, Wu, Wd, ln_w, cls_W, cls_b, meta)
    res = bass_utils.run_bass_kernel_spmd(nc, in_maps, core_ids=list(range(8)))
    return np.concatenate(
        [np.asarray(res.results[k]["out"], np.float32) for k in range(8)],
        axis=0)
